# revision 9
# baseline (speedup 1.0000x reference)
"""Trainium2 Bass kernel for ArcShapeRadiusConfigVisibleNeighDist.

For each pedestrian i (N=8192):
  heading u_i = normalize(pos_i - past_i)
  over all j: dist_ij = |pos_j - pos_i|, visible iff angle(pos_j-pos_i, u_i)
  in [-35deg, 35deg) and j != i. Output = affine(clip(mean visible dist)).

Key reformulation (no atan2 anywhere):
  visible  <=>  rel . u_i > cos(35deg) * dist  <=>  dot/c > dist
  sq and dot/c are K-small matmuls on the TensorEngine with fp16 hi/lo
  split features (K is free on the PE), giving near-fp32 accuracy at
  full PE speed. G1 (K=10, rows 0-9) and G2 (K=8, rows 32-39) are packed
  into different PE row-groups via tile_position so they run concurrently.

Per 128-query x 1024-j chunk (single fused vector pass per element):
  PE:  G1 = sq (+eps) [128,1024], G2 = dot/c [128,1024]
  ACT: dist = sqrt(G1) -> fp16 [128,1024]
  DVE: custom MASKED_SDC: b = select(G2 > dist, dist + ENC_C, 0),
       accum -> A = ENC_C*cnt + s  (one accumulator carries BOTH the
       visible count and the visible-distance sum; per-chunk s < 2^17
       so the host separates them with a floor-divide).
Host epilogue: cnt = floor((A+64)/ENC_C); s = A - ENC_C*cnt per chunk,
  summed over chunks; r = clip(s/max(cnt,1) * k + b, 0.5, 4.0);
  select by indexes.

Sharding: core k owns queries [k*1024, (k+1)*1024), full j set.
"""

import numpy as np

import concourse.bass as bass
import concourse.bacc as bacc
import concourse.mybir as mybir
import concourse.tile as tile
from contextlib import ExitStack
from concourse.bass_utils import run_bass_kernel_spmd
from concourse.dve_uop import DveOpSpec
import concourse.dve_ops as dvo
from concourse.dve_ops import Spec, Src0, Src1, Zero, C1, select, lower, has_src1
from concourse.dve_ops import AluOp as SAluOp

N = 8192
NCORES = 8
Q = N // NCORES            # 1024 queries per core
ITILES = Q // 128          # 8 partition tiles of queries
JCHUNK = 1024
NJC = N // JCHUNK          # 8 j-chunks per i-tile
EPS = 0.005                # sq guard: keeps diag excluded, sqrt input > 0
COS_HALF = float(np.cos(70.0 * np.pi / 180.0 / 2.0))
MIN_R, MAX_R = 0.5, 4.0
MIN_D, MAX_D = 0.2, 5.0
SLOPE = (MAX_R - MIN_R) / (MAX_D - MIN_D)
OFFS = MIN_R - MIN_D * SLOPE
ENC_C = 131072.0           # 2^17: per-chunk s < 1024*dmax ~ 98e3 < 2^17

F32 = mybir.dt.float32
FP16 = mybir.dt.float16
ACTF = mybir.ActivationFunctionType
_F16 = np.float16

JF_SPLIT = 4               # jf DMA'd as column tiles so first matmuls start early


def register_masked_sdc():
    """Runtime-register the fused DVE op:
    out = select(in0 > in1, in1 + s1, 0), accum_out = sum(out).
    With s1 = ENC_C the accumulator encodes ENC_C*count + sum(dist) in one
    fp32 lane. The per-NEFF uop table is generated from OPS, so appending
    at runtime is sufficient (no firmware change)."""
    name = "MASKED_SDC_ANT"
    if name in dvo._SUB_OPCODE_FOR_NAME:
        return getattr(dvo, name)

    def _ref(in0, in1, s0, s1, imm2):
        b = np.where(in0.astype(np.float32) > in1,
                     in1.astype(np.float32) + np.float32(s1),
                     0.0).astype(np.float32)
        return b, b.reshape(b.shape[0], -1).sum(axis=-1, keepdims=True)

    spec = Spec(body=select(Src0 > Src1, Src1 + C1, Zero), accum=SAluOp.ADD,
                reference=_ref)
    row = max(dvo._SUB_OPCODE_FOR_NAME.values()) + 1
    assert row < 0x20
    dvo._SUB_OPCODE_FOR_NAME[name] = row
    op = dvo.DveOp(name, spec, subdim=False, uops_sha={})
    for ver in ("v3", "v4"):
        s = DveOpSpec(name=name, opcode=row, uops=lower(spec, ver=ver),
                      rd1_en=has_src1(spec))
        op.uops_sha[ver] = s.sha(ver)
    dvo.OPS.append(op)
    dvo.CUSTOM_DVE_SPECS[name] = spec
    setattr(dvo, name, op)
    return op


def _split(x):
    """Split f64 array into fp16 hi + fp16 lo (as f64 of exact fp16 values)."""
    h = x.astype(_F16).astype(np.float64)
    l = (x - h).astype(_F16).astype(np.float64)
    return h, l


def _build_graph():
    masked_sdc = register_masked_sdc()
    nc = bacc.Bacc("TRN2", target_bir_lowering=False, debug=False,
                   num_devices=NCORES)
    # rows 0-9: G1 features; rows 32-39: G2 features (row-group 1).
    # head* pack qf with the first jf column block so one DMA descriptor
    # unblocks the first matmuls (Sync descriptor writes are ~1us each).
    jw = N // JF_SPLIT
    h1_d = nc.dram_tensor("h1", [10, Q + jw], FP16, kind="ExternalInput")
    h2_d = nc.dram_tensor("h2", [8, Q + jw], FP16, kind="ExternalInput")
    t1_d = nc.dram_tensor("t1", [10, N - jw], FP16, kind="ExternalInput")
    t2_d = nc.dram_tensor("t2", [8, N - jw], FP16, kind="ExternalInput")
    oa_d = nc.dram_tensor("out_a", [128, ITILES * NJC], F32,
                          kind="ExternalOutput")

    with tile.TileContext(nc) as tc, ExitStack() as ctx:
        singles = ctx.enter_context(tc.tile_pool(name="singles", bufs=1))
        psum = ctx.enter_context(tc.tile_pool(name="psum", bufs=2, space="PSUM"))
        work = ctx.enter_context(tc.tile_pool(name="work", bufs=4))

        # separate tiles per PE row-group so the G1 chain never waits on a
        # G2-row DMA landing in the same tile (startup critical path)
        ha = singles.tile([10, Q + jw], FP16)
        hb = singles.tile([40, Q + jw], FP16)  # rows 32-39 used
        ta = singles.tile([10, N - jw], FP16)
        tb = singles.tile([40, N - jw], FP16)  # rows 32-39 used
        nc.sync.dma_start(ha[:], h1_d[:])
        nc.sync.dma_start(hb[32:40, :], h2_d[:])
        nc.sync.dma_start(ta[:], t1_d[:])
        nc.sync.dma_start(tb[32:40, :], t2_d[:])
        # single-writer accumulator stripes; final math happens on host.
        # Two tiles so most of the output DMA is issued mid-kernel.
        HC = ITILES * NJC // 2
        a_lo = singles.tile([128, HC], F32)
        a_hi = singles.tile([128, HC], F32)

        for it in range(ITILES):
            lhs1 = ha[:, bass.ts(it, 128)]
            lhs2 = hb[32:40, bass.ts(it, 128)]
            for jc in range(NJC):
                gi = it * NJC + jc
                g1 = psum.tile([128, JCHUNK], F32, tag="g1")
                g2 = psum.tile([128, JCHUNK], F32, tag="g2")
                # all G1 matmuls before G2's: the G1->sqrt chain unblocks
                # on the first input DMA alone
                for grp in range(2):
                    for h in range(2):
                        col = jc * JCHUNK + h * 512
                        hs = slice(h * 512, (h + 1) * 512)
                        if col < jw:
                            s1, s2 = ha, hb
                            cl = Q + col
                        else:
                            s1, s2 = ta, tb
                            cl = col - jw
                        if grp == 0:
                            nc.tensor.matmul(g1[:, hs], lhs1,
                                             s1[:, cl:cl + 512],
                                             tile_position=(0, 0))
                        else:
                            nc.tensor.matmul(g2[:, hs], lhs2,
                                             s2[32:40, cl:cl + 512],
                                             tile_position=(32, 0))
                dist = work.tile([128, JCHUNK], FP16, tag="dist")
                nc.scalar.activation(dist[:], g1[:], ACTF.Sqrt)
                junk = work.tile([128, JCHUNK], mybir.dt.float8e4, tag="jk")
                a_t = a_lo if gi < HC else a_hi
                nc.vector._custom_dve(
                    masked_sdc, out=junk[:], in0=g2[:], in1=dist[:],
                    s1=ENC_C, accum_out=a_t[:, gi % HC:gi % HC + 1])
            if it == ITILES // 2 - 1:
                nc.sync.dma_start(oa_d[:, 0:HC], a_lo[:])

        nc.sync.dma_start(oa_d[:, HC:], a_hi[:])

    nc.compile()
    return nc


_CACHED_NC = None


def _get_nc():
    global _CACHED_NC
    if _CACHED_NC is None:
        _CACHED_NC = _build_graph()
    return _CACHED_NC


def _prep_inputs(past_ped_positions, ped_positions, indexes, all_radii):
    pos = np.asarray(ped_positions, np.float64)
    past = np.asarray(past_ped_positions, np.float64)
    v = pos - past
    vn = np.hypot(v[:, 0], v[:, 1])
    safe = np.where(vn > 0, vn, 1.0)
    ux = np.where(vn > 0, v[:, 0] / safe, 1.0)
    uy = np.where(vn > 0, v[:, 1] / safe, 0.0)

    px, py = pos[:, 0], pos[:, 1]
    nsq = px * px + py * py
    px_h, px_l = _split(px)
    py_h, py_l = _split(py)
    nsq_h, nsq_l = _split(nsq)
    ones = np.ones(N)
    jf1 = np.stack([px_h, px_l, px_h, py_h, py_l, py_h, ones, ones,
                    nsq_h, nsq_l]).astype(_F16)
    jf2 = jf1[0:8].copy()

    a = ux / COS_HALF
    b = uy / COS_HALF
    w = (ux * px + uy * py) / COS_HALF
    a_h, a_l = _split(a)
    b_h, b_l = _split(b)
    w_h, w_l = _split(w)
    nq_h, nq_l = _split(nsq + EPS)
    qf1_full = np.stack([-2 * px_h, -2 * px_h, -2 * px_l,
                         -2 * py_h, -2 * py_h, -2 * py_l,
                         nq_h, nq_l, ones, ones])  # [10, N]
    qf2_full = np.stack([a_h, a_h, a_l, b_h, b_h, b_l, -w_h, -w_l])  # [8, N]

    # column c of per-core qf holds local query (c % 128) * ITILES + c // 128
    cidx = np.arange(Q)
    perm = (cidx % 128) * ITILES + cidx // 128

    jw = N // JF_SPLIT
    t1 = np.ascontiguousarray(jf1[:, jw:])
    t2 = np.ascontiguousarray(jf2[:, jw:])
    in_maps = []
    for k in range(NCORES):
        sl = slice(k * Q, (k + 1) * Q)
        qf1_core = qf1_full[:, sl][:, perm].astype(_F16)
        qf2_core = qf2_full[:, sl][:, perm].astype(_F16)
        h1 = np.concatenate([qf1_core, jf1[:, :jw]], axis=1)
        h2 = np.concatenate([qf2_core, jf2[:, :jw]], axis=1)
        in_maps.append({"h1": h1, "h2": h2, "t1": t1, "t2": t2})
    return in_maps


def _host_epilogue(res_core, idxf_core, radii_core):
    """[128, 64] encoded accumulator stripes -> [1024] final radii for one
    core. idxf_core/radii_core are [128, ITILES] (local query
    q = p*ITILES + it). Each accumulator lane holds ENC_C*cnt + s for one
    (query, j-chunk); s < ENC_C so floor-divide separates them (+64 absorbs
    downward fp accumulation error in near-empty chunks)."""
    A = np.asarray(res_core["out_a"], np.float64).reshape(128, ITILES, NJC)
    cnt_c = np.floor((A + 64.0) / ENC_C)
    s_c = A - ENC_C * cnt_c
    c = cnt_c.sum(2)
    s = s_c.sum(2)
    mean = (s / np.maximum(c, 1.0)).astype(np.float32)
    r = np.clip(mean * np.float32(SLOPE) + np.float32(OFFS), MIN_R, MAX_R)
    fin = radii_core + idxf_core * (r - radii_core)
    return fin.astype(np.float32).reshape(Q)


def kernel(past_ped_positions, ped_positions, indexes, all_radii,
           _trace=False, _trace_kwargs=None):
    nc = _get_nc()
    in_maps = _prep_inputs(past_ped_positions, ped_positions, indexes,
                           all_radii)
    kw = {}
    if _trace:
        kw = {"trace": True}
        if _trace_kwargs:
            kw.update(_trace_kwargs)
    res = run_bass_kernel_spmd(nc, in_maps, list(range(NCORES)), **kw)
    idxf = np.asarray(indexes).astype(np.float32)
    radii = np.asarray(all_radii, np.float32)
    outs = []
    for k in range(NCORES):
        sl = slice(k * Q, (k + 1) * Q)
        outs.append(_host_epilogue(res.results[k],
                                   idxf[sl].reshape(128, ITILES),
                                   radii[sl].reshape(128, ITILES)))
    out = np.concatenate(outs)
    if _trace:
        kernel.last_results = res
    return out


# revision 14
# speedup vs baseline: 1.2184x; 1.2184x over previous
"""Trainium2 Bass kernel for ArcShapeRadiusConfigVisibleNeighDist.

For each pedestrian i (N=8192):
  heading u_i = normalize(pos_i - past_i)
  over all j: dist_ij = |pos_j - pos_i|, visible iff angle(pos_j-pos_i, u_i)
  in [-35deg, 35deg) and j != i. Output = affine(clip(mean visible dist)).

Key reformulation (no atan2 anywhere):
  visible  <=>  rel . u_i > cos(35deg) * dist  <=>  dot/c > dist
  sq and dot/c are K-small matmuls on the TensorEngine with fp16 hi/lo
  split features (K is free on the PE), giving near-fp32 accuracy at
  full PE speed. G1 (K=10, rows 0-9) and G2 (K=8, rows 32-39) are packed
  into different PE row-groups via tile_position so they run concurrently.

Per 128-query x 1024-j chunk (single fused vector pass per element):
  PE:  G1 = sq (+eps) [128,1024], G2 = dot/c [128,1024]
  ACT: dist = sqrt(G1) -> fp16 [128,1024]
  DVE: custom MASKED_SDC: b = select(G2 > dist, dist + ENC_C, 0),
       accum -> A = ENC_C*cnt + s  (one accumulator carries BOTH the
       visible count and the visible-distance sum; per-chunk s < 2^17
       so the host separates them with a floor-divide).
Host epilogue: cnt = floor((A+64)/ENC_C); s = A - ENC_C*cnt per chunk,
  summed over chunks; r = clip(s/max(cnt,1) * k + b, 0.5, 4.0);
  select by indexes.

Sharding: core k owns queries [k*1024, (k+1)*1024), full j set.
"""

import numpy as np

import concourse.bass as bass
import concourse.bacc as bacc
import concourse.mybir as mybir
import concourse.tile as tile
from contextlib import ExitStack
from concourse.bass_utils import run_bass_kernel_spmd
from concourse.dve_uop import DveOpSpec
import concourse.dve_ops as dvo
from concourse.dve_ops import Spec, Src0, Src1, Zero, C1, select, lower, has_src1
from concourse.dve_ops import AluOp as SAluOp

N = 8192
NCORES = 8
Q = N // NCORES            # 1024 queries per core
ITILES = Q // 128          # 8 partition tiles of queries
JCHUNK = 1024
NJC = N // JCHUNK          # 8 j-chunks per i-tile
EPS = 0.005                # sq guard: keeps diag excluded, sqrt input > 0
COS_HALF = float(np.cos(70.0 * np.pi / 180.0 / 2.0))
MIN_R, MAX_R = 0.5, 4.0
MIN_D, MAX_D = 0.2, 5.0
SLOPE = (MAX_R - MIN_R) / (MAX_D - MIN_D)
OFFS = MIN_R - MIN_D * SLOPE
ENC_C = 131072.0           # 2^17: per-chunk s < 1024*dmax ~ 98e3 < 2^17

F32 = mybir.dt.float32
FP16 = mybir.dt.float16
ACTF = mybir.ActivationFunctionType
_F16 = np.float16

JF_SPLIT = 4               # jf DMA'd as column tiles so first matmuls start early


def register_masked_sdc():
    """Runtime-register the fused DVE op:
    out = select(in0 > in1, in1 + s1, 0), accum_out = sum(out).
    With s1 = ENC_C the accumulator encodes ENC_C*count + sum(dist) in one
    fp32 lane. The per-NEFF uop table is generated from OPS, so appending
    at runtime is sufficient (no firmware change)."""
    name = "MASKED_SDC_ANT"
    if name in dvo._SUB_OPCODE_FOR_NAME:
        return getattr(dvo, name)

    def _ref(in0, in1, s0, s1, imm2):
        b = np.where(in1.astype(np.float32) > in0,
                     in0.astype(np.float32) + np.float32(s1),
                     0.0).astype(np.float32)
        return b, b.reshape(b.shape[0], -1).sum(axis=-1, keepdims=True)

    # dist rides in0 (SBUF), the PSUM operand rides in1: the inter-op
    # read-write bubble tracks rd0, and SBUF-src is the cheaper one
    spec = Spec(body=select(Src1 > Src0, Src0 + C1, Zero), accum=SAluOp.ADD,
                reference=_ref)
    row = max(dvo._SUB_OPCODE_FOR_NAME.values()) + 1
    assert row < 0x20
    dvo._SUB_OPCODE_FOR_NAME[name] = row
    op = dvo.DveOp(name, spec, subdim=False, uops_sha={})
    for ver in ("v3", "v4"):
        s = DveOpSpec(name=name, opcode=row, uops=lower(spec, ver=ver),
                      rd1_en=has_src1(spec))
        op.uops_sha[ver] = s.sha(ver)
    dvo.OPS.append(op)
    dvo.CUSTOM_DVE_SPECS[name] = spec
    setattr(dvo, name, op)
    return op


def _split(x):
    """Split f64 array into fp16 hi + fp16 lo (as f64 of exact fp16 values)."""
    h = x.astype(_F16).astype(np.float64)
    l = (x - h).astype(_F16).astype(np.float64)
    return h, l


def _build_graph():
    masked_sdc = register_masked_sdc()
    nc = bacc.Bacc("TRN2", target_bir_lowering=False, debug=False,
                   num_devices=NCORES)
    # rows 0-9: G1 features; rows 32-39: G2 features (row-group 1).
    # head* pack qf with the first jf column block so one DMA descriptor
    # unblocks the first matmuls (Sync descriptor writes are ~1us each).
    jw = N // JF_SPLIT
    h1_d = nc.dram_tensor("h1", [10, Q + jw], FP16, kind="ExternalInput")
    h2_d = nc.dram_tensor("h2", [8, Q + jw], FP16, kind="ExternalInput")
    t1_d = nc.dram_tensor("t1", [10, N - jw], FP16, kind="ExternalInput")
    t2_d = nc.dram_tensor("t2", [8, N - jw], FP16, kind="ExternalInput")
    TSPL = jw  # tail chunk boundary: cols [0, TSPL) land first
    oa_d = nc.dram_tensor("out_a", [128, ITILES * NJC], F32,
                          kind="ExternalOutput")

    with tile.TileContext(nc) as tc, ExitStack() as ctx:
        singles = ctx.enter_context(tc.tile_pool(name="singles", bufs=1))
        psum = ctx.enter_context(tc.tile_pool(name="psum", bufs=2, space="PSUM"))
        work = ctx.enter_context(tc.tile_pool(name="work", bufs=4))

        # separate tiles per PE row-group so the G1 chain never waits on a
        # G2-row DMA landing in the same tile (startup critical path)
        ha = singles.tile([10, Q + jw], FP16)
        hb = singles.tile([40, Q + jw], FP16)  # rows 32-39 used
        ta0 = singles.tile([10, TSPL], FP16)
        tb0 = singles.tile([40, TSPL], FP16)  # rows 32-39 used
        ta1 = singles.tile([10, N - 2 * jw], FP16)
        tb1 = singles.tile([40, N - 2 * jw], FP16)  # rows 32-39 used
        nc.sync.dma_start(ha[:], h1_d[:])
        nc.sync.dma_start(hb[32:40, :], h2_d[:])
        nc.sync.dma_start(ta0[:], t1_d[:, 0:TSPL])
        nc.sync.dma_start(tb0[32:40, :], t2_d[:, 0:TSPL])
        nc.sync.dma_start(ta1[:], t1_d[:, TSPL:])
        nc.sync.dma_start(tb1[32:40, :], t2_d[:, TSPL:])
        # single-writer accumulator stripes; final math happens on host.
        # Two tiles so most of the output DMA is issued mid-kernel.
        HC = ITILES * NJC // 2
        a_lo = singles.tile([128, HC], F32)
        a_hi = singles.tile([128, HC], F32)

        for it in range(ITILES):
            lhs1 = ha[:, bass.ts(it, 128)]
            lhs2 = hb[32:40, bass.ts(it, 128)]
            for jc in range(NJC):
                gi = it * NJC + jc
                g1 = psum.tile([128, JCHUNK], F32, tag="g1")
                g2 = psum.tile([128, JCHUNK], F32, tag="g2")
                # all G1 matmuls before G2's: the G1->sqrt chain unblocks
                # on the first input DMA alone
                for grp in range(2):
                    for h in range(2):
                        col = jc * JCHUNK + h * 512
                        hs = slice(h * 512, (h + 1) * 512)
                        if col < jw:
                            s1, s2 = ha, hb
                            cl = Q + col
                        elif col < jw + TSPL:
                            s1, s2 = ta0, tb0
                            cl = col - jw
                        else:
                            s1, s2 = ta1, tb1
                            cl = col - jw - TSPL
                        if grp == 0:
                            nc.tensor.matmul(g1[:, hs], lhs1,
                                             s1[:, cl:cl + 512],
                                             tile_position=(0, 0))
                        else:
                            nc.tensor.matmul(g2[:, hs], lhs2,
                                             s2[32:40, cl:cl + 512],
                                             tile_position=(32, 0))
                dist = work.tile([128, JCHUNK], FP16, tag="dist")
                nc.scalar.activation(dist[:], g1[:], ACTF.Sqrt)
                junk = work.tile([128, JCHUNK], mybir.dt.float8e4, tag="jk")
                a_t = a_lo if gi < HC else a_hi
                nc.vector._custom_dve(
                    masked_sdc, out=junk[:], in0=dist[:], in1=g2[:],
                    s1=ENC_C, accum_out=a_t[:, gi % HC:gi % HC + 1])
            if it == ITILES // 2 - 1:
                nc.sync.dma_start(oa_d[:, 0:HC], a_lo[:])

        nc.sync.dma_start(oa_d[:, HC:], a_hi[:])

    nc.compile()
    return nc


_CACHED_NC = None


def _get_nc():
    global _CACHED_NC
    if _CACHED_NC is None:
        _CACHED_NC = _build_graph()
    return _CACHED_NC


def _prep_inputs(past_ped_positions, ped_positions, indexes, all_radii):
    pos = np.asarray(ped_positions, np.float64)
    past = np.asarray(past_ped_positions, np.float64)
    v = pos - past
    vn = np.hypot(v[:, 0], v[:, 1])
    safe = np.where(vn > 0, vn, 1.0)
    ux = np.where(vn > 0, v[:, 0] / safe, 1.0)
    uy = np.where(vn > 0, v[:, 1] / safe, 0.0)

    px, py = pos[:, 0], pos[:, 1]
    nsq = px * px + py * py
    px_h, px_l = _split(px)
    py_h, py_l = _split(py)
    nsq_h, nsq_l = _split(nsq)
    ones = np.ones(N)
    jf1 = np.stack([px_h, px_l, px_h, py_h, py_l, py_h, ones, ones,
                    nsq_h, nsq_l]).astype(_F16)
    jf2 = jf1[0:8].copy()

    a = ux / COS_HALF
    b = uy / COS_HALF
    w = (ux * px + uy * py) / COS_HALF
    a_h, a_l = _split(a)
    b_h, b_l = _split(b)
    w_h, w_l = _split(w)
    nq_h, nq_l = _split(nsq + EPS)
    qf1_full = np.stack([-2 * px_h, -2 * px_h, -2 * px_l,
                         -2 * py_h, -2 * py_h, -2 * py_l,
                         nq_h, nq_l, ones, ones])  # [10, N]
    qf2_full = np.stack([a_h, a_h, a_l, b_h, b_h, b_l, -w_h, -w_l])  # [8, N]

    # column c of per-core qf holds local query (c % 128) * ITILES + c // 128
    cidx = np.arange(Q)
    perm = (cidx % 128) * ITILES + cidx // 128

    jw = N // JF_SPLIT
    t1 = np.ascontiguousarray(jf1[:, jw:])
    t2 = np.ascontiguousarray(jf2[:, jw:])
    in_maps = []
    for k in range(NCORES):
        sl = slice(k * Q, (k + 1) * Q)
        qf1_core = qf1_full[:, sl][:, perm].astype(_F16)
        qf2_core = qf2_full[:, sl][:, perm].astype(_F16)
        h1 = np.concatenate([qf1_core, jf1[:, :jw]], axis=1)
        h2 = np.concatenate([qf2_core, jf2[:, :jw]], axis=1)
        in_maps.append({"h1": h1, "h2": h2, "t1": t1, "t2": t2})
    return in_maps


def _host_epilogue(res_core, idxf_core, radii_core):
    """[128, 64] encoded accumulator stripes -> [1024] final radii for one
    core. idxf_core/radii_core are [128, ITILES] (local query
    q = p*ITILES + it). Each accumulator lane holds ENC_C*cnt + s for one
    (query, j-chunk); s < ENC_C so floor-divide separates them (+64 absorbs
    downward fp accumulation error in near-empty chunks)."""
    A = np.asarray(res_core["out_a"], np.float64).reshape(128, ITILES, NJC)
    cnt_c = np.floor((A + 64.0) / ENC_C)
    s_c = A - ENC_C * cnt_c
    c = cnt_c.sum(2)
    s = s_c.sum(2)
    mean = (s / np.maximum(c, 1.0)).astype(np.float32)
    r = np.clip(mean * np.float32(SLOPE) + np.float32(OFFS), MIN_R, MAX_R)
    fin = radii_core + idxf_core * (r - radii_core)
    return fin.astype(np.float32).reshape(Q)


def kernel(past_ped_positions, ped_positions, indexes, all_radii,
           _trace=False, _trace_kwargs=None):
    nc = _get_nc()
    in_maps = _prep_inputs(past_ped_positions, ped_positions, indexes,
                           all_radii)
    kw = {}
    if _trace:
        kw = {"trace": True}
        if _trace_kwargs:
            kw.update(_trace_kwargs)
    res = run_bass_kernel_spmd(nc, in_maps, list(range(NCORES)), **kw)
    idxf = np.asarray(indexes).astype(np.float32)
    radii = np.asarray(all_radii, np.float32)
    outs = []
    for k in range(NCORES):
        sl = slice(k * Q, (k + 1) * Q)
        outs.append(_host_epilogue(res.results[k],
                                   idxf[sl].reshape(128, ITILES),
                                   radii[sl].reshape(128, ITILES)))
    out = np.concatenate(outs)
    if _trace:
        kernel.last_results = res
    return out


# revision 16
# speedup vs baseline: 1.2270x; 1.0071x over previous
"""Trainium2 Bass kernel for ArcShapeRadiusConfigVisibleNeighDist.

For each pedestrian i (N=8192):
  heading u_i = normalize(pos_i - past_i)
  over all j: dist_ij = |pos_j - pos_i|, visible iff angle(pos_j-pos_i, u_i)
  in [-35deg, 35deg) and j != i. Output = affine(clip(mean visible dist)).

Key reformulation (no atan2 anywhere):
  visible  <=>  rel . u_i > cos(35deg) * dist  <=>  dot/c > dist
  sq and dot/c are K-small matmuls on the TensorEngine with fp16 hi/lo
  split features (K is free on the PE), giving near-fp32 accuracy at
  full PE speed. G1 (K=10, rows 0-9) and G2 (K=8, rows 32-39) are packed
  into different PE row-groups via tile_position so they run concurrently.

Per 128-query x 1024-j chunk (single fused vector pass per element):
  PE:  G1 = sq (+eps) [128,1024], G2 = dot/c [128,1024]
  ACT: dist = sqrt(G1) -> fp16 [128,1024]
  DVE: custom MASKED_SDC: b = select(G2 > dist, dist + ENC_C, 0),
       accum -> A = ENC_C*cnt + s  (one accumulator carries BOTH the
       visible count and the visible-distance sum; per-chunk s < 2^17
       so the host separates them with a floor-divide).
Host epilogue: cnt = floor((A+64)/ENC_C); s = A - ENC_C*cnt per chunk,
  summed over chunks; r = clip(s/max(cnt,1) * k + b, 0.5, 4.0);
  select by indexes.

Sharding: core k owns queries [k*1024, (k+1)*1024), full j set.
"""

import numpy as np

import concourse.bass as bass
import concourse.bacc as bacc
import concourse.mybir as mybir
import concourse.tile as tile
from contextlib import ExitStack
from concourse.bass_utils import run_bass_kernel_spmd
from concourse.dve_uop import DveOpSpec
import concourse.dve_ops as dvo
from concourse.dve_ops import Spec, Src0, Src1, Zero, C1, select, lower, has_src1
from concourse.dve_ops import AluOp as SAluOp

N = 8192
NCORES = 8
Q = N // NCORES            # 1024 queries per core
ITILES = Q // 128          # 8 partition tiles of queries
JCHUNK = 1024
NJC = N // JCHUNK          # 8 j-chunks per i-tile
EPS = 0.005                # sq guard: keeps diag excluded, sqrt input > 0
COS_HALF = float(np.cos(70.0 * np.pi / 180.0 / 2.0))
MIN_R, MAX_R = 0.5, 4.0
MIN_D, MAX_D = 0.2, 5.0
SLOPE = (MAX_R - MIN_R) / (MAX_D - MIN_D)
OFFS = MIN_R - MIN_D * SLOPE
ENC_C = 131072.0           # 2^17: per-chunk s < 1024*dmax ~ 98e3 < 2^17

F32 = mybir.dt.float32
FP16 = mybir.dt.float16
ACTF = mybir.ActivationFunctionType
_F16 = np.float16

JF_SPLIT = 4               # jf DMA'd as column tiles so first matmuls start early


def register_masked_sdc():
    """Runtime-register the fused DVE op:
    out = select(in0 > in1, in1 + s1, 0), accum_out = sum(out).
    With s1 = ENC_C the accumulator encodes ENC_C*count + sum(dist) in one
    fp32 lane. The per-NEFF uop table is generated from OPS, so appending
    at runtime is sufficient (no firmware change)."""
    name = "MASKED_SDC_ANT"
    if name in dvo._SUB_OPCODE_FOR_NAME:
        return getattr(dvo, name)

    def _ref(in0, in1, s0, s1, imm2):
        b = np.where(in1.astype(np.float32) > in0,
                     in0.astype(np.float32) + np.float32(s1),
                     0.0).astype(np.float32)
        return b, b.reshape(b.shape[0], -1).sum(axis=-1, keepdims=True)

    # dist rides in0 (SBUF), the PSUM operand rides in1: the inter-op
    # read-write bubble tracks rd0, and SBUF-src is the cheaper one
    spec = Spec(body=select(Src1 > Src0, Src0 + C1, Zero), accum=SAluOp.ADD,
                reference=_ref)
    row = max(dvo._SUB_OPCODE_FOR_NAME.values()) + 1
    assert row < 0x20
    dvo._SUB_OPCODE_FOR_NAME[name] = row
    op = dvo.DveOp(name, spec, subdim=False, uops_sha={})
    for ver in ("v3", "v4"):
        s = DveOpSpec(name=name, opcode=row, uops=lower(spec, ver=ver),
                      rd1_en=has_src1(spec))
        op.uops_sha[ver] = s.sha(ver)
    dvo.OPS.append(op)
    dvo.CUSTOM_DVE_SPECS[name] = spec
    setattr(dvo, name, op)
    return op


def _split(x):
    """Split f64 array into fp16 hi + fp16 lo (as f64 of exact fp16 values)."""
    h = x.astype(_F16).astype(np.float64)
    l = (x - h).astype(_F16).astype(np.float64)
    return h, l


def _build_graph():
    masked_sdc = register_masked_sdc()
    nc = bacc.Bacc("TRN2", target_bir_lowering=False, debug=False,
                   num_devices=NCORES)
    # rows 0-9: G1 features; rows 32-39: G2 features (row-group 1).
    # head* pack qf with the first jf column block so one DMA descriptor
    # unblocks the first matmuls (Sync descriptor writes are ~1us each).
    jw = N // JF_SPLIT
    h1_d = nc.dram_tensor("h1", [10, Q + jw], FP16, kind="ExternalInput")
    h2_d = nc.dram_tensor("h2", [8, Q + jw], FP16, kind="ExternalInput")
    t1_d = nc.dram_tensor("t1", [10, N - jw], FP16, kind="ExternalInput")
    t2_d = nc.dram_tensor("t2", [8, N - jw], FP16, kind="ExternalInput")
    TSPL = jw  # tail chunk boundary: cols [0, TSPL) land first
    oa_d = nc.dram_tensor("out_a", [128, ITILES * NJC], F32,
                          kind="ExternalOutput")

    with tile.TileContext(nc) as tc, ExitStack() as ctx:
        singles = ctx.enter_context(tc.tile_pool(name="singles", bufs=1))
        psum = ctx.enter_context(tc.tile_pool(name="psum", bufs=2, space="PSUM"))
        work = ctx.enter_context(tc.tile_pool(name="work", bufs=4))

        # separate tiles per PE row-group so the G1 chain never waits on a
        # G2-row DMA landing in the same tile (startup critical path)
        ha = singles.tile([10, Q + jw], FP16)
        hb = singles.tile([40, Q + jw], FP16)  # rows 32-39 used
        TH = (N - 2 * jw) // 2
        ta0 = singles.tile([10, TSPL], FP16)
        tb0 = singles.tile([40, TSPL], FP16)  # rows 32-39 used
        ta1 = singles.tile([10, TH], FP16)
        tb1 = singles.tile([40, TH], FP16)  # rows 32-39 used
        ta2 = singles.tile([10, TH], FP16)
        tb2 = singles.tile([40, TH], FP16)  # rows 32-39 used
        nc.sync.dma_start(ha[:], h1_d[:])
        nc.sync.dma_start(hb[32:40, :], h2_d[:])
        nc.sync.dma_start(ta0[:], t1_d[:, 0:TSPL])
        nc.sync.dma_start(tb0[32:40, :], t2_d[:, 0:TSPL])
        nc.sync.dma_start(ta1[:], t1_d[:, TSPL:TSPL + TH])
        nc.sync.dma_start(tb1[32:40, :], t2_d[:, TSPL:TSPL + TH])
        nc.sync.dma_start(ta2[:], t1_d[:, TSPL + TH:])
        nc.sync.dma_start(tb2[32:40, :], t2_d[:, TSPL + TH:])
        # single-writer accumulator stripes; final math happens on host.
        # Two tiles so most of the output DMA is issued mid-kernel.
        HC = ITILES * NJC // 2
        a_lo = singles.tile([128, HC], F32)
        a_hi = singles.tile([128, HC], F32)

        for it in range(ITILES):
            lhs1 = ha[:, bass.ts(it, 128)]
            lhs2 = hb[32:40, bass.ts(it, 128)]
            for jc in range(NJC):
                gi = it * NJC + jc
                g1 = psum.tile([128, JCHUNK], F32, tag="g1")
                g2 = psum.tile([128, JCHUNK], F32, tag="g2")
                # all G1 matmuls before G2's: the G1->sqrt chain unblocks
                # on the first input DMA alone
                for grp in range(2):
                    for h in range(2):
                        col = jc * JCHUNK + h * 512
                        hs = slice(h * 512, (h + 1) * 512)
                        if col < jw:
                            s1, s2 = ha, hb
                            cl = Q + col
                        elif col < jw + TSPL:
                            s1, s2 = ta0, tb0
                            cl = col - jw
                        elif col < jw + TSPL + TH:
                            s1, s2 = ta1, tb1
                            cl = col - jw - TSPL
                        else:
                            s1, s2 = ta2, tb2
                            cl = col - jw - TSPL - TH
                        if grp == 0:
                            nc.tensor.matmul(g1[:, hs], lhs1,
                                             s1[:, cl:cl + 512],
                                             tile_position=(0, 0))
                        else:
                            nc.tensor.matmul(g2[:, hs], lhs2,
                                             s2[32:40, cl:cl + 512],
                                             tile_position=(32, 0))
                dist = work.tile([128, JCHUNK], FP16, tag="dist")
                nc.scalar.activation(dist[:], g1[:], ACTF.Sqrt)
                junk = work.tile([128, JCHUNK], mybir.dt.float8e4, tag="jk")
                a_t = a_lo if gi < HC else a_hi
                nc.vector._custom_dve(
                    masked_sdc, out=junk[:], in0=dist[:], in1=g2[:],
                    s1=ENC_C, accum_out=a_t[:, gi % HC:gi % HC + 1])
            if it == ITILES // 2 - 1:
                nc.sync.dma_start(oa_d[:, 0:HC], a_lo[:])

        nc.sync.dma_start(oa_d[:, HC:], a_hi[:])

    nc.compile()
    return nc


_CACHED_NC = None


def _get_nc():
    global _CACHED_NC
    if _CACHED_NC is None:
        _CACHED_NC = _build_graph()
    return _CACHED_NC


def _prep_inputs(past_ped_positions, ped_positions, indexes, all_radii):
    pos = np.asarray(ped_positions, np.float64)
    past = np.asarray(past_ped_positions, np.float64)
    v = pos - past
    vn = np.hypot(v[:, 0], v[:, 1])
    safe = np.where(vn > 0, vn, 1.0)
    ux = np.where(vn > 0, v[:, 0] / safe, 1.0)
    uy = np.where(vn > 0, v[:, 1] / safe, 0.0)

    px, py = pos[:, 0], pos[:, 1]
    nsq = px * px + py * py
    px_h, px_l = _split(px)
    py_h, py_l = _split(py)
    nsq_h, nsq_l = _split(nsq)
    ones = np.ones(N)
    jf1 = np.stack([px_h, px_l, px_h, py_h, py_l, py_h, ones, ones,
                    nsq_h, nsq_l]).astype(_F16)
    jf2 = jf1[0:8].copy()

    a = ux / COS_HALF
    b = uy / COS_HALF
    w = (ux * px + uy * py) / COS_HALF
    a_h, a_l = _split(a)
    b_h, b_l = _split(b)
    w_h, w_l = _split(w)
    nq_h, nq_l = _split(nsq + EPS)
    qf1_full = np.stack([-2 * px_h, -2 * px_h, -2 * px_l,
                         -2 * py_h, -2 * py_h, -2 * py_l,
                         nq_h, nq_l, ones, ones])  # [10, N]
    qf2_full = np.stack([a_h, a_h, a_l, b_h, b_h, b_l, -w_h, -w_l])  # [8, N]

    # column c of per-core qf holds local query (c % 128) * ITILES + c // 128
    cidx = np.arange(Q)
    perm = (cidx % 128) * ITILES + cidx // 128

    jw = N // JF_SPLIT
    t1 = np.ascontiguousarray(jf1[:, jw:])
    t2 = np.ascontiguousarray(jf2[:, jw:])
    in_maps = []
    for k in range(NCORES):
        sl = slice(k * Q, (k + 1) * Q)
        qf1_core = qf1_full[:, sl][:, perm].astype(_F16)
        qf2_core = qf2_full[:, sl][:, perm].astype(_F16)
        h1 = np.concatenate([qf1_core, jf1[:, :jw]], axis=1)
        h2 = np.concatenate([qf2_core, jf2[:, :jw]], axis=1)
        in_maps.append({"h1": h1, "h2": h2, "t1": t1, "t2": t2})
    return in_maps


def _host_epilogue(res_core, idxf_core, radii_core):
    """[128, 64] encoded accumulator stripes -> [1024] final radii for one
    core. idxf_core/radii_core are [128, ITILES] (local query
    q = p*ITILES + it). Each accumulator lane holds ENC_C*cnt + s for one
    (query, j-chunk); s < ENC_C so floor-divide separates them (+64 absorbs
    downward fp accumulation error in near-empty chunks)."""
    A = np.asarray(res_core["out_a"], np.float64).reshape(128, ITILES, NJC)
    cnt_c = np.floor((A + 64.0) / ENC_C)
    s_c = A - ENC_C * cnt_c
    c = cnt_c.sum(2)
    s = s_c.sum(2)
    mean = (s / np.maximum(c, 1.0)).astype(np.float32)
    r = np.clip(mean * np.float32(SLOPE) + np.float32(OFFS), MIN_R, MAX_R)
    fin = radii_core + idxf_core * (r - radii_core)
    return fin.astype(np.float32).reshape(Q)


def kernel(past_ped_positions, ped_positions, indexes, all_radii,
           _trace=False, _trace_kwargs=None):
    nc = _get_nc()
    in_maps = _prep_inputs(past_ped_positions, ped_positions, indexes,
                           all_radii)
    kw = {}
    if _trace:
        kw = {"trace": True}
        if _trace_kwargs:
            kw.update(_trace_kwargs)
    res = run_bass_kernel_spmd(nc, in_maps, list(range(NCORES)), **kw)
    idxf = np.asarray(indexes).astype(np.float32)
    radii = np.asarray(all_radii, np.float32)
    outs = []
    for k in range(NCORES):
        sl = slice(k * Q, (k + 1) * Q)
        outs.append(_host_epilogue(res.results[k],
                                   idxf[sl].reshape(128, ITILES),
                                   radii[sl].reshape(128, ITILES)))
    out = np.concatenate(outs)
    if _trace:
        kernel.last_results = res
    return out
